# revision 1
# baseline (speedup 1.0000x reference)
"""Bass/Trainium2 SPMD kernel for nn_MultiHeadAttention_83923660964113.

Reference model: B=2, N=5000, C=256, H=8 heads, dk=32, M=N//2=2500
  q   = x @ Wq.T + bq                       -> (B, H, N, dk)
  x_  = LayerNorm(einsum('mn,bnc->bmc', Wc, x) + bc; gamma, beta)
  k,v = x_ @ Wk.T + bk, x_ @ Wv.T + bv      -> (B, H, M, dk)
  out = softmax(q k^T / sqrt(dk)) v         -> (B, N, C)

Sharding over 8 cores (SPMD, one program; all per-core differences live in
input *data*, never in addresses):
  core c: batch b=c//4, shard j=c%4.  Phase 1 computes x_n^T for m rows
  [625j, 625j+625) of its batch (the 12.8-GFLOP Wc matmul sharded 8 ways),
  then a 4-core-group AllGather exchanges x_n^T shards (group-relative slot
  order makes addressing batch-independent).  Each core then projects
  K^T/V/Q for its two heads (2j, 2j+1) from host-sliced weight rows and runs
  dense attention.

Performance structure:
  - all matmuls in float32r: fp32 data, FP22 multiply, fp32 accumulate,
    full PE rate at output free-dim >= 256 (no bf16 conversions, ~1e-4 errs)
  - scores computed transposed, S^T[m, q] (row-tiled K=32 matmuls, 2 PSUM
    banks wide), so softmax exp output P^T is directly the PV matmul rhs --
    no transposes of the 200M-element probability matrix
  - softmax skips max-subtraction (scores are O(1): |s| < ~2, exact in fp32);
    1/sqrt(dk) and the LN affine (gamma/beta) are folded into weights on host
  - PV is col-tiled; a ones-column weight emits the softmax denominator for
    free; the final small transpose back to [q, dv] applies 1/denominator as
    a per-partition scale during the drain
  - exp on ScalarE at 1024 elems/instruction is the critical path (~25M
    elements/core); PE/DVE/DMA run underneath it
"""

import os
import sys

for _p in ("/opt/trn_rl_repo", "/root/.axon_site/_ro/trn_rl_repo"):
    if os.path.isdir(_p) and _p not in sys.path:
        sys.path.insert(0, _p)

import contextlib

import numpy as np

import concourse.bass as bass
import concourse.bacc as bacc
import concourse.tile as tile
from concourse import mybir

F32 = mybir.dt.float32
F32R = mybir.dt.float32r
AF = mybir.ActivationFunctionType

B, N, C, H, DK = 2, 5000, 256, 8, 32
M = N // 2            # 2500 kv positions
MS = M // 4           # 625 per-core m-shard
MT = 125              # m tile (5/shard, 20 total)
NT = 125              # n (contract) tile for the Wc matmul: 40 tiles
QC = 512              # q chunk for attention (10 chunks: 9x512 + 392)
EPS = 1e-5
KV_SZ = C * MS        # 160000 floats: one x_n^T shard [256, 625]

_CACHE: dict = {}


BISECT = os.environ.get("KBISECT", "full")


def _build(has_bv: bool):
    nc = bacc.Bacc(
        "TRN2",
        target_bir_lowering=False,
        debug=False,
        enable_asserts=False,
        num_devices=8,
    )

    din = {}
    for name, shape in [
        ("xb", [N, C]),          # x[b]
        ("xbt", [C, N]),         # x[b].T
        ("wct", [N, MS]),        # Wc[m-shard rows].T
        ("bc", [MS]),
        ("wqt", [C, 64]),        # (Wq[head rows]/sqrt(dk)).T
        ("bq", [64]),
        ("wkt", [C, 64]),        # (Wk*gamma)[head rows].T
        ("bk", [64]),
        ("wvt", [C, 64]),        # (Wv*gamma)[head rows].T
        ("bv", [64]),
        ("ident", [128, 128]),
    ]:
        din[name] = nc.dram_tensor(name, shape, F32, kind="ExternalInput").ap()
    out_d = nc.dram_tensor("out", [N, 64], F32, kind="ExternalOutput").ap()

    kv_loc = nc.dram_tensor("kv_loc", [KV_SZ], F32).ap()
    kv_all = nc.dram_tensor("kv_all", [4, KV_SZ], F32).ap()

    with tile.TileContext(nc) as tc:
        _emit(tc, din, out_d, kv_loc, kv_all, has_bv)
    nc.compile()
    return nc


def _emit(tc, din, out_d, kv_loc, kv_all, has_bv):
    nc = tc.nc

    with contextlib.ExitStack() as ctx:
        consts = ctx.enter_context(tc.tile_pool(name="consts", bufs=1))
        stream = ctx.enter_context(tc.tile_pool(name="stream", bufs=3))
        mid = ctx.enter_context(tc.tile_pool(name="mid", bufs=1))

        # ---- constants / small weights ----
        ident = consts.tile([128, 128], F32)
        nc.sync.dma_start(ident, din["ident"])
        eps_t = consts.tile([MT, 1], F32)
        nc.vector.memset(eps_t, EPS)

        wqt = consts.tile([128, 2, 64], F32R)   # [c-part, ct, head col]
        nc.sync.dma_start(wqt, din["wqt"].rearrange("(t p) k -> p t k", p=128).bitcast(F32R))
        wkt = consts.tile([128, 2, 64], F32R)
        nc.sync.dma_start(wkt, din["wkt"].rearrange("(t p) k -> p t k", p=128).bitcast(F32R))
        wvt = consts.tile([128, 2, 64], F32R)
        nc.sync.dma_start(wvt, din["wvt"].rearrange("(t p) k -> p t k", p=128).bitcast(F32R))
        bc_t = consts.tile([MT, 5], F32)
        nc.sync.dma_start(bc_t, din["bc"].rearrange("(t p) -> p t", p=MT))
        bk_t = consts.tile([64, 1], F32)
        nc.sync.dma_start(bk_t, din["bk"].rearrange("(k o) -> k o", o=1))
        bq_t = consts.tile([64, 1], F32)
        nc.sync.dma_start(bq_t, din["bq"].rearrange("(k o) -> k o", o=1))

        qt_rep = mid.tile([128, N], F32R)       # Q^T row-replicated: [A,A,B,B] x 32
        krep = mid.tile([128, M], F32R)         # K^T row-replicated: [A,A,B,B] x 32
        v_sb = mid.tile([MT, 20, 66], F32R)     # V m-tiles: [1 | V_A | 1 | V_B]
        xnt_all = mid.tile([128, 2, M], F32R)   # gathered x_n^T [2 c-tiles, 2500]
        nc.vector.memset(v_sb[:, :, 0].bitcast(F32), 1.0)
        nc.vector.memset(v_sb[:, :, 33].bitcast(F32), 1.0)

        # =================  Phase 1: x_ = Wc@x, LN, x_n^T shard  =================
        with tc.tile_pool(name="px", bufs=5, space="PSUM") as px, \
             tc.tile_pool(name="pt", bufs=2, space="PSUM") as pt:

            x_ps = [px.tile([MT, C], F32, tag="xps", name=f"x_ps{i}")
                    for i in range(5)]
            for kt in range(N // NT):
                wct_t = stream.tile([NT, MS], F32R, tag="wct")
                nc.sync.dma_start(wct_t, din["wct"][NT * kt:NT * (kt + 1), :].bitcast(F32R))
                xb_t = stream.tile([NT, C], F32R, tag="xb")
                nc.sync.dma_start(xb_t, din["xb"][NT * kt:NT * (kt + 1), :].bitcast(F32R))
                for mt in range(5):
                    nc.tensor.matmul(
                        x_ps[mt],
                        lhsT=wct_t[:, MT * mt:MT * (mt + 1)],
                        rhs=xb_t,
                        start=(kt == 0), stop=(kt == N // NT - 1),
                    )

            # LayerNorm (normalize only; affine folded into Wk/Wv host-side)
            x_sb = mid.tile([MT, 5, C], F32)
            xn_sb = mid.tile([MT, 5, C], F32)
            sq_sb = mid.tile([MT, C], F32, tag="sq")
            s1 = mid.tile([MT, 5], F32)
            s2 = mid.tile([MT, 5], F32)
            mean = mid.tile([MT, 5], F32)
            var = mid.tile([MT, 5], F32)
            rstd = mid.tile([MT, 5], F32)
            nmr = mid.tile([MT, 5], F32)
            for mt in range(5):
                m1 = slice(mt, mt + 1)
                nc.scalar.activation(x_sb[:, mt, :], x_ps[mt], AF.Identity,
                                     bias=bc_t[:, m1], scale=1.0,
                                     accum_out=s1[:, m1])
                nc.scalar.activation(sq_sb, x_sb[:, mt, :], AF.Square,
                                     accum_out=s2[:, m1])
                nc.vector.tensor_scalar_mul(mean[:, m1], s1[:, m1], 1.0 / C)
                nc.vector.tensor_scalar_mul(var[:, m1], s2[:, m1], 1.0 / C)
                nc.vector.tensor_mul(nmr[:, m1], mean[:, m1], mean[:, m1])
                nc.vector.tensor_tensor(out=var[:, m1], in0=var[:, m1],
                                        in1=nmr[:, m1], op=mybir.AluOpType.subtract)
                nc.scalar.activation(rstd[:, m1], var[:, m1], AF.Sqrt,
                                     bias=eps_t, scale=1.0)
                nc.vector.reciprocal(rstd[:, m1], rstd[:, m1])
                nc.vector.tensor_mul(nmr[:, m1], mean[:, m1], rstd[:, m1])
                nc.vector.tensor_scalar_mul(nmr[:, m1], nmr[:, m1], -1.0)
                nc.scalar.activation(xn_sb[:, mt, :], x_sb[:, mt, :], AF.Identity,
                                     bias=nmr[:, m1], scale=rstd[:, m1])

            # transpose x_n -> x_n^T [2 c-tiles x 128, 625], ship shard to DRAM
            xnt = mid.tile([128, 2, MS], F32R)
            for mt in range(5):
                for ct in range(2):
                    tp = pt.tile([128, MT], F32, tag="tp")
                    nc.tensor.transpose(tp, xn_sb[:, mt, 128 * ct:128 * (ct + 1)],
                                        ident[0:MT, 0:MT])
                    nc.vector.tensor_copy(xnt[:, ct, MT * mt:MT * (mt + 1)], tp)
            nc.sync.dma_start(
                kv_loc.rearrange("(p t m) -> p t m", t=2, m=MS).bitcast(F32R), xnt)

            # Q^T [64, 5000] = (Wq/sqrt(dk))[heads] @ x[b]^T (+bq)
            for i in range(10):
                qo = 500 * i
                qps = pt.tile([64, 500], F32, tag="tp")
                for ct in range(2):
                    xbt_t = stream.tile([128, 500], F32R, tag="xbt")
                    nc.sync.dma_start(
                        xbt_t,
                        din["xbt"][128 * ct:128 * (ct + 1), qo:qo + 500].bitcast(F32R))
                    nc.tensor.matmul(
                        qps,
                        lhsT=wqt[:, ct, :],
                        rhs=xbt_t,
                        start=(ct == 0), stop=(ct == 1),
                    )
                qtmp = mid.tile([64, 500], F32R, tag="qtmp", name="qtmp")
                nc.scalar.activation(qtmp[0:32, :], qps[0:32, :],
                                     AF.Identity, bias=bq_t[0:32], scale=1.0)
                nc.vector.tensor_scalar_add(qtmp[32:64, :],
                                            qps[32:64, :], bq_t[32:64])
                nc.sync.dma_start(qt_rep[0:32, qo:qo + 500], qtmp[0:32, :])
                nc.sync.dma_start(qt_rep[32:64, qo:qo + 500], qtmp[0:32, :])
                nc.sync.dma_start(qt_rep[64:96, qo:qo + 500], qtmp[32:64, :])
                nc.sync.dma_start(qt_rep[96:128, qo:qo + 500], qtmp[32:64, :])

            # =================  AllGather x_n^T within batch group  =================
            if BISECT != "nogather":
                nc.gpsimd.collective_compute(
                    "AllGather", mybir.AluOpType.bypass,
                    replica_groups=[[0, 1, 2, 3], [4, 5, 6, 7]],
                    ins=[kv_loc],
                    outs=[kv_all],
                )
                gather_src = kv_all
            else:
                gather_src = None
            for i in range(4):
                if gather_src is not None:
                    src_ap = gather_src[i, :]
                else:
                    src_ap = kv_loc
                nc.sync.dma_start(
                    xnt_all[:, :, MS * i:MS * (i + 1)],
                    src_ap.rearrange("(p t m) -> p t m", t=2, m=MS).bitcast(F32R))

            # K^T for our 2 heads, row-replicated for QK^T row-tiling:
            #   krep rows [0:32]=A, [32:64]=A, [64:96]=B, [96:128]=B
            for mo in range(0, M, 512):
                mw = min(512, M - mo)
                kps = pt.tile([64, 512], F32, tag="tp")
                for ct in range(2):
                    nc.tensor.matmul(
                        kps[:, 0:mw],
                        lhsT=wkt[:, ct, :],
                        rhs=xnt_all[:, ct, mo:mo + mw],
                        start=(ct == 0), stop=(ct == 1),
                    )
                ktmp = mid.tile([64, 512], F32R, tag="ktmp", name="ktmp")
                nc.scalar.activation(ktmp[0:32, 0:mw], kps[0:32, 0:mw],
                                     AF.Identity, bias=bk_t[0:32], scale=1.0)
                nc.vector.tensor_scalar_add(ktmp[32:64, 0:mw],
                                            kps[32:64, 0:mw], bk_t[32:64])
                nc.sync.dma_start(krep[0:32, mo:mo + mw], ktmp[0:32, 0:mw])
                nc.sync.dma_start(krep[32:64, mo:mo + mw], ktmp[0:32, 0:mw])
                nc.sync.dma_start(krep[64:96, mo:mo + mw], ktmp[32:64, 0:mw])
                nc.sync.dma_start(krep[96:128, mo:mo + mw], ktmp[32:64, 0:mw])

            # V m-tiles [125, 64] for our 2 heads (+bv if nonzero)
            bvb = None
            if has_bv:
                bv_src = din["bv"]
                bvb = consts.tile([MT, 64], F32)
                nc.sync.dma_start(
                    bvb, bass.AP(tensor=bv_src.tensor, offset=bv_src.offset,
                                 ap=[[0, MT]] + [list(p) for p in bv_src.ap]))
            for mt in range(20):
                vps = px.tile([MT, 64], F32, tag="xps")
                for ct in range(2):
                    nc.tensor.matmul(
                        vps,
                        lhsT=xnt_all[:, ct, MT * mt:MT * (mt + 1)],
                        rhs=wvt[:, ct, :],
                        start=(ct == 0), stop=(ct == 1),
                    )
                if has_bv:
                    nc.vector.tensor_add(v_sb[:, mt, 1:33], vps[:, 0:32], bvb[:, 0:32])
                    nc.vector.tensor_add(v_sb[:, mt, 34:66], vps[:, 32:64], bvb[:, 32:64])
                else:
                    nc.vector.tensor_copy(v_sb[:, mt, 1:33], vps[:, 0:32])
                    nc.vector.tensor_copy(v_sb[:, mt, 34:66], vps[:, 32:64])

        # ==========================  Attention  ==========================
        # per (q-chunk, head): 10 rounds of 2 row-tiled QK^T -> exp -> 2
        # col-tiled PV (accumulating out^T and denominator in one PSUM bank)
        with tc.tile_pool(name="ps_s", bufs=2, space="PSUM") as ps_s, \
             tc.tile_pool(name="ps_o", bufs=2, space="PSUM") as ps_o, \
             tc.tile_pool(name="psb", bufs=3) as psb, \
             tc.tile_pool(name="tail", bufs=2) as tail:
            if BISECT == "noattn":
                zero_sb = tail.tile([128, 64], F32, tag="outsb", name="zero_sb")
                nc.vector.memset(zero_sb, 0.0)
                for qt_i in range(0, N, 128):
                    qn = min(128, N - qt_i)
                    nc.sync.dma_start(out_d[qt_i:qt_i + qn, :], zero_sb[0:qn, :])
            attn_ranges = [] if BISECT == "noattn" else list(range(0, N, QC))
            for qc in attn_ranges:
                cw = min(QC, N - qc)
                o_ps2 = [ps_o.tile([33, QC], F32, tag="ops", name=f"o_ps{hh}")
                         for hh in range(2)]
                for h in range(2):
                    o_ps = o_ps2[h]
                    # head h: psum row 0 = denom, rows [1:33] = out^T
                    for u in range(10):
                        s_ps = ps_s.tile([MT, 1024], F32, tag="sps")
                        for g in range(2):
                            mt = 2 * u + g
                            nc.tensor.matmul(
                                s_ps[:, 512 * g:512 * g + cw],
                                lhsT=krep[64 * h + 32 * g:64 * h + 32 * (g + 1),
                                          MT * mt:MT * (mt + 1)],
                                rhs=qt_rep[64 * h + 32 * g:64 * h + 32 * (g + 1),
                                           qc:qc + cw],
                                start=True, stop=True,
                                tile_position=(64 * h + 32 * g, 0),
                            )
                        p_sb = psb.tile([MT, 1024], F32R, tag="psb")
                        if cw == QC:
                            nc.scalar.activation(p_sb, s_ps, AF.Exp)
                        else:
                            for g in range(2):
                                nc.scalar.activation(
                                    p_sb[:, 512 * g:512 * g + cw],
                                    s_ps[:, 512 * g:512 * g + cw], AF.Exp)
                        for g in range(2):
                            mt = 2 * u + g
                            nc.tensor.matmul(
                                o_ps[:, 0:cw],
                                lhsT=v_sb[:, mt, 33 * h:33 * (h + 1)],
                                rhs=p_sb[:, 512 * g:512 * g + cw],
                                start=(u == 0 and g == 0), stop=(u == 9 and g == 1),
                            )

                # tail: transpose back to [q, dv], normalize by 1/denom, store
                o_sbA = tail.tile([33, QC], F32, tag="osbA")
                o_sbB = tail.tile([33, QC], F32, tag="osbB")
                nc.vector.tensor_copy(o_sbA[:, 0:cw], o_ps2[0][:, 0:cw])
                nc.vector.tensor_copy(o_sbB[:, 0:cw], o_ps2[1][:, 0:cw])
                for s in range(0, cw, 128):
                    qs = min(128, cw - s)
                    t_ps = ps_o.tile([128, 128], F32, tag="tps")
                    nc.tensor.transpose(t_ps[0:qs, 0:33], o_sbA[:, s:s + qs],
                                        ident[0:33, 0:33])
                    nc.tensor.transpose(t_ps[0:qs, 64:97], o_sbB[:, s:s + qs],
                                        ident[0:33, 0:33])
                    rden = tail.tile([128, 2], F32, tag="rden")
                    nc.vector.reciprocal(rden[0:qs, :], t_ps[0:qs, 0:128:64])
                    out_sb = tail.tile([128, 64], F32, tag="outsb")
                    nc.vector.tensor_scalar_mul(out_sb[0:qs, 0:32],
                                                t_ps[0:qs, 1:33], rden[0:qs, 0:1])
                    nc.vector.tensor_scalar_mul(out_sb[0:qs, 32:64],
                                                t_ps[0:qs, 65:97], rden[0:qs, 1:2])
                    nc.sync.dma_start(out_d[qc + s:qc + s + qs, :], out_sb[0:qs, :])


def _prep_core_inputs(c, x, Wq, bq, Wk, bk, Wv, bv, Wc, bc, gamma, beta):
    b, j = c // 4, c % 4
    hc = slice(64 * j, 64 * j + 64)      # head rows/cols for heads 2j, 2j+1
    sc = np.float32(1.0 / np.sqrt(DK))
    wq_eff = Wq[hc, :] * sc
    bq_eff = bq[hc] * sc
    wk_eff = (Wk * gamma[None, :])[hc, :]
    bk_eff = (Wk @ beta + bk)[hc]
    wv_eff = (Wv * gamma[None, :])[hc, :]
    bv_eff = (Wv @ beta + bv)[hc]
    out = {
        "xb": x[b],
        "xbt": x[b].T,
        "wct": Wc[MS * j:MS * (j + 1), :].T,
        "bc": bc[MS * j:MS * (j + 1)],
        "wqt": wq_eff.T,
        "bq": bq_eff,
        "wkt": wk_eff.T,
        "bk": bk_eff,
        "wvt": wv_eff.T,
        "bv": bv_eff,
        "ident": np.eye(128, dtype=np.float32),
    }
    return {k: np.ascontiguousarray(v, dtype=np.float32) for k, v in out.items()}


def kernel(x, Wq, bq, Wk, bk, Wv, bv, Wc, bc, gamma, beta, **_unused):
    x = np.asarray(x, np.float32)
    Wq, bq = np.asarray(Wq, np.float32), np.asarray(bq, np.float32)
    Wk, bk = np.asarray(Wk, np.float32), np.asarray(bk, np.float32)
    Wv, bv = np.asarray(Wv, np.float32), np.asarray(bv, np.float32)
    Wc, bc = np.asarray(Wc, np.float32), np.asarray(bc, np.float32)
    gamma, beta = np.asarray(gamma, np.float32), np.asarray(beta, np.float32)

    has_bv = bool(np.any(Wv @ beta + bv))
    key = ("nc", has_bv)
    if key not in _CACHE:
        _CACHE[key] = _build(has_bv)
    nc = _CACHE[key]

    in_maps = [
        _prep_core_inputs(c, x, Wq, bq, Wk, bk, Wv, bv, Wc, bc, gamma, beta)
        for c in range(8)
    ]

    results = _run_cached(nc, in_maps)

    out = np.empty((B, N, C), np.float32)
    for c in range(8):
        b, j = c // 4, c % 4
        out[b, :, 64 * j:64 * j + 64] = results[c]["out"]
    return out


def _run_cached(nc, in_maps, n_cores=8):
    """Like bass2jax.run_bass_via_pjrt but caches the jitted executable so
    repeated kernel() calls skip retracing/recompiling."""
    import jax
    from jax.sharding import Mesh, PartitionSpec
    from jax.experimental.shard_map import shard_map
    import concourse.mybir as mybir_
    from concourse import bass2jax

    if "exec" not in _CACHE:
        bass2jax.install_neuronx_cc_hook()
        pid_name = nc.partition_id_tensor.name if nc.partition_id_tensor else None
        in_names, out_names, out_avals, zero_outs = [], [], [], []
        for alloc in nc.m.functions[0].allocations:
            if not isinstance(alloc, mybir_.MemoryLocationSet):
                continue
            name = alloc.memorylocations[0].name
            if alloc.kind == "ExternalInput":
                if name != pid_name:
                    in_names.append(name)
            elif alloc.kind == "ExternalOutput":
                out_names.append(name)
                shape = tuple(alloc.tensor_shape)
                dtype = mybir_.dt.np(alloc.dtype)
                out_avals.append(jax.core.ShapedArray(shape, dtype))
                zero_outs.append(np.zeros(shape, dtype))
        n_params = len(in_names)
        all_names = in_names + out_names
        if pid_name is not None:
            all_names = all_names + [pid_name]
        donate = tuple(range(n_params, n_params + len(out_names)))

        def _body(*args):
            operands = list(args)
            if pid_name is not None:
                operands.append(bass2jax.partition_id_tensor())
            outs = bass2jax._bass_exec_p.bind(
                *operands,
                out_avals=tuple(out_avals),
                in_names=tuple(all_names),
                out_names=tuple(out_names),
                lowering_input_output_aliases=(),
                sim_require_finite=True,
                sim_require_nnan=True,
                nc=nc,
            )
            return tuple(outs)

        devices = jax.devices()[:n_cores]
        mesh = Mesh(np.asarray(devices), ("core",))
        specs = (PartitionSpec("core"),)
        sharded = jax.jit(
            shard_map(_body, mesh=mesh,
                      in_specs=specs * (n_params + len(out_names)),
                      out_specs=specs * len(out_names), check_rep=False),
            donate_argnums=donate, keep_unused=True)
        _CACHE["exec"] = (sharded, in_names, out_names, zero_outs)

    sharded, in_names, out_names, zero_outs = _CACHE["exec"]
    concat_in = [
        np.concatenate([np.asarray(in_maps[c][n]) for c in range(n_cores)], axis=0)
        for n in in_names
    ]
    concat_zero = [np.concatenate([z] * n_cores, axis=0) for z in zero_outs]
    out_arrs = sharded(*concat_in, *concat_zero)
    out_arrs = [np.asarray(a) for a in out_arrs]
    results = []
    for c in range(n_cores):
        d = {}
        for i, nm in enumerate(out_names):
            per = out_arrs[i].shape[0] // n_cores
            d[nm] = out_arrs[i][c * per:(c + 1) * per]
        results.append(d)
    return results


if __name__ == "__main__":
    # smoke test against local jax reference if available
    sys.path.insert(0, "/root/problem")
    import reference

    inputs = {k: np.asarray(v) for k, v in reference.setup_inputs().items()}
    expected = np.asarray(reference.reference(**inputs))
    actual = kernel(**inputs)
    err = np.linalg.norm(actual - expected) / np.linalg.norm(expected)
    print("Relative error:", err)

